# revision 15
# baseline (speedup 1.0000x reference)
"""LSTM (H=32, input-size 1) over B=32, T=16384 on 8 TRN2 NeuronCores.

Strategy: pure data parallel over batch (4 rows per core). Within a core,
the sequence is split into 2 HALVES ("streams") processed concurrently —
the second half starts from a zero state one chunk early, and those
warm-up outputs are discarded (LSTM state memory here is ~40 steps, so a
256-step warm-up is exact to fp32 noise; verified against the reference).
The two streams' dependency chains interleave on the engines, hiding the
serial latency of each.

Within a stream, the recurrence is evaluated chunk-by-chunk with Jacobi
(Picard) sweeps — DEER-style parallel-in-time evaluation:

  * chunk of K timesteps, J fixed-point sweeps per chunk
  * gate pre-activations accumulate in PSUM:  raw += W_bd @ dH  (the
    stationary operand is a block-diagonal [128,128] replication of the
    32x32 per-gate recurrent weight over the 4 local batch rows, so ONE
    matmul per gate covers all batches and lands directly in the
    (batch,hidden)-partition layout used by the elementwise engines)
  * g-gate weights are pre-scaled by 2 host-side, so ONE sigmoid covers
    all four gates of a stream (tanh(g) = 2*sigmoid(2g)-1); the
    correction folds into the fused (sig_g - 0.5)*sig_i DVE op, which
    computes m/2 — the c recurrence then runs at half scale and
    tanh(c) = tanh(2*(c/2)) uses the activation's free input scale
  * the c-recurrence c_t = f_t*c_{t-1} + m_t over a whole chunk is ONE
    DVE tensor_tensor_scan instruction per stream
  * convergence is geometric (~10x per sweep) and chunk-size independent
    (measured), so J=8 reaches ~2e-5.

Everything (weight block-diagonalization, gate reorder to [i,f,o,g],
bias folding into the x-injection matmul) is precomputed host-side.
"""

import os
import numpy as np

import concourse.bass as bass
import concourse.bacc as bacc
import concourse.tile as tile
import concourse.mybir as mybir
from concourse.bass_utils import run_bass_kernel_spmd

H = 32
B = 32
T = 16384
NCORES = 8
BL = B // NCORES          # batch rows per core = 4
P = BL * H                # 128 partitions = (batch, hidden)
NS = 2                    # streams (sequence halves) per core

K = int(os.environ.get("LSTM_K", "256"))    # chunk length per stream
J = int(os.environ.get("LSTM_J", "8"))      # Jacobi sweeps per chunk
MM = os.environ.get("LSTM_MM", "f32r")      # matmul operand dtype: f32r | f32

F32 = mybir.dt.float32
F32R = mybir.dt.float32r
MMDT = F32R if MM == "f32r" else F32
AF = mybir.ActivationFunctionType
OP = mybir.AluOpType


def build_nc(k=K, j_iters=J, t_total=T):
    nc = bacc.Bacc("TRN2", target_bir_lowering=False, debug=False)

    half = t_total // NS
    assert half % k == 0
    n_chunks = half // k + 1              # +1 warm-up/pad chunk per stream
    # stream 0 covers [0, half + k) (last chunk's y discarded)
    # stream 1 covers [half - k, t_total) (first chunk's y discarded)
    GW = 4 * k                            # raw columns per stream

    x_d = nc.declare_dram_parameter("x", [BL, t_total], MMDT, isOutput=False)
    wbd_d = nc.declare_dram_parameter("wbd", [P, 4 * P], MMDT, isOutput=False)
    rj_d = nc.declare_dram_parameter("rj", [2 * BL, 4 * P], MMDT, isOutput=False)
    wo_d = nc.declare_dram_parameter("wo", [P, BL], MMDT, isOutput=False)
    bo_d = nc.declare_dram_parameter("bo", [BL, 1], F32, isOutput=False)
    y_d = nc.declare_dram_parameter("y", [BL, t_total], F32, isOutput=True)

    def xcol(r, n):
        """first x column of stream r, chunk n (may exceed t_total for the
        stream-0 pad chunk: clamped; its y is discarded anyway)"""
        base = n * k if r == 0 else half - k + n * k
        return min(base, t_total - k)

    def keep_y(r, n):
        return (n < n_chunks - 1) if r == 0 else (n > 0)

    with tile.TileContext(nc) as tc:
        with (
            tc.tile_pool(name="const", bufs=1) as cpool,
            tc.tile_pool(name="state", bufs=1) as spool,
            tc.tile_pool(name="work", bufs=3) as wpool,
            tc.tile_pool(name="praw", bufs=1, space="PSUM") as praw,
            tc.tile_pool(name="py", bufs=2, space="PSUM") as pypool,
        ):
            # ---- constants ----
            wbd = cpool.tile([P, 4 * P], MMDT)
            rj = cpool.tile([2 * BL, 4 * P], MMDT)
            wo = cpool.tile([P, BL], MMDT)
            bo = cpool.tile([BL, 1], F32)
            zrow = cpool.tile([1, P], MMDT)
            nc.vector.memset(zrow[:].bitcast(F32), 0.0)
            nc.sync.dma_start(wbd[:], wbd_d[:])
            nc.sync.dma_start(rj[:], rj_d[:])
            nc.sync.dma_start(wo[:], wo_d[:])
            nc.sync.dma_start(bo[:], bo_d[:])

            # ---- persistent state (per stream blocks) ----
            hbufs = [spool.tile([P, NS * (k + 1)], MMDT, tag=t, name=t)
                     for t in ("hA", "hB")]
            dlt = spool.tile([P, NS * k], MMDT)
            ccar = spool.tile([P, NS], F32)

            nc.vector.memset(hbufs[0][:].bitcast(F32), 0.0)
            nc.vector.memset(hbufs[1][:].bitcast(F32), 0.0)
            nc.vector.memset(ccar[:], 0.0)

            # PSUM raw gates: per stream [i|f|o|g], k cols each
            raw = praw.tile([P, NS * GW], F32)

            def blk(r, g):
                return slice(r * GW + g * k, r * GW + (g + 1) * k)

            def hcols(buf, r):                 # h value cols (excl carry col)
                return buf[:, r * (k + 1) + 1 : r * (k + 1) + 1 + k]

            def hprev(buf, r):                 # shifted view incl carry col
                return buf[:, r * (k + 1) : r * (k + 1) + k]

            for n in range(n_chunks):
                # ---- per-chunk input: X rows (x_b at 2b, ones at 2b+1) ----
                xt = wpool.tile([2 * BL, NS * k], MMDT, tag="xt")
                nc.vector.memset(xt[:].bitcast(F32), 1.0)
                for r in range(NS):
                    c0 = xcol(r, n)
                    for b in range(BL):
                        nc.sync.dma_start(
                            xt[2 * b : 2 * b + 1, r * k : (r + 1) * k],
                            x_d[b : b + 1, c0 : c0 + k])

                if n > 0:
                    nc.gpsimd.memset(hcols(hbufs[0], 0).bitcast(F32), 0.0)
                    nc.gpsimd.memset(hcols(hbufs[0], 1).bitcast(F32), 0.0)

                # ---- zero raw banks (only start=True writes; full banks) ----
                for bk in range(NS * GW // 512):
                    nc.tensor.matmul(
                        raw[:, bk * 512 : (bk + 1) * 512],
                        zrow[:], wbd[0:1, 0:512],
                        start=True, stop=False, skip_group_check=True)

                # ---- x/bias injection: raw += Rg^T @ X ----
                for r in range(NS):
                    for g in range(4):
                        nc.tensor.matmul(
                            raw[:, blk(r, g)],
                            rj[:, g * P : (g + 1) * P],
                            xt[:, r * k : (r + 1) * k],
                            start=False, stop=False, skip_group_check=True)

                # ---- Jacobi sweeps ----
                for j in range(1, j_iters + 1):
                    gbuf = hbufs[(j - 1) % 2]
                    nbuf = hbufs[j % 2]

                    sig = wpool.tile([P, NS * GW], F32, tag="sig")
                    c = wpool.tile([P, NS * k], F32, tag="c")
                    m = wpool.tile([P, NS * k], F32, tag="m")
                    tau = wpool.tile([P, NS * k], F32, tag="tau")

                    for r in range(NS):
                        rhs = hprev(gbuf, r) if j == 1 else dlt[:, r * k : (r + 1) * k]
                        for g in range(4):
                            nc.tensor.matmul(
                                raw[:, blk(r, g)],
                                wbd[:, g * P : (g + 1) * P],
                                rhs,
                                start=False, stop=(j == j_iters),
                                skip_group_check=True)

                        # one sigmoid over [i|f|o|2g] of this stream
                        nc.scalar.activation(
                            sig[:, r * GW : (r + 1) * GW],
                            raw[:, r * GW : (r + 1) * GW], AF.Sigmoid)

                        i_s, f_s, o_s, sg_s = (sig[:, blk(r, g)] for g in range(4))
                        m_s = m[:, r * k : (r + 1) * k]
                        c_s = c[:, r * k : (r + 1) * k]
                        tau_s = tau[:, r * k : (r + 1) * k]

                        # m/2 = (sig(2g) - 0.5) * sig(i)   [tanh folded]
                        nc.vector.scalar_tensor_tensor(
                            m_s, sg_s, 0.5, i_s, OP.subtract, OP.mult)

                        nc.vector.tensor_tensor_scan(
                            c_s, f_s, m_s, ccar[:, r : r + 1], OP.mult, OP.add)

                        # tanh(c) = tanh(2 * (c/2)) via free input scale
                        nc.scalar.activation(tau_s, c_s, AF.Tanh, scale=2.0)

                        nc.vector.tensor_mul(hcols(nbuf, r), o_s, tau_s)

                        if j < j_iters:
                            nc.vector.tensor_sub(
                                dlt[:, r * k : (r + 1) * k],
                                hprev(nbuf, r), hprev(gbuf, r))

                fin = hbufs[j_iters % 2]

                # ---- output projection y = W_out @ h + b_out ----
                yp = pypool.tile([BL, NS * k], F32)
                for r in range(NS):
                    nc.tensor.matmul(
                        yp[:, r * k : (r + 1) * k], wo[:], hcols(fin, r),
                        start=True, stop=True)
                ysb = wpool.tile([BL, NS * k], F32, tag="ysb")
                nc.scalar.activation(ysb[:], yp[:], AF.Identity, bias=bo[:])
                for r in range(NS):
                    if keep_y(r, n):
                        c0 = xcol(r, n)
                        nc.sync.dma_start(
                            y_d[:, c0 : c0 + k], ysb[:, r * k : (r + 1) * k])

                # ---- carries for next chunk ----
                if n < n_chunks - 1:
                    for r in range(NS):
                        last = fin[:, r * (k + 1) + k : r * (k + 1) + k + 1]
                        nc.vector.tensor_copy(
                            hbufs[0][:, r * (k + 1) : r * (k + 1) + 1], last)
                        nc.vector.tensor_copy(
                            hbufs[1][:, r * (k + 1) : r * (k + 1) + 1], last)
                        nc.vector.tensor_copy(
                            ccar[:, r : r + 1],
                            c[:, r * k + k - 1 : r * k + k])

    nc.compile()
    return nc


def _host_precompute(W_ih, W_hh, b_ih, b_hh, W_out, b_out):
    """Block-diagonal stationary operands; gate order -> [i,f,o,g];
    g-gate rows pre-scaled by 2 (tanh-via-sigmoid folding)."""
    perm = np.concatenate([np.arange(0, 32), np.arange(32, 64),
                           np.arange(96, 128), np.arange(64, 96)])
    scale = np.ones((128, 1), np.float32)
    scale[96:] = 2.0                      # g block doubled
    Wh = W_hh[perm] * scale               # (128, 32)
    Wi = (W_ih[perm, 0:1] * scale)[:, 0]  # (128,)
    bs = (b_ih + b_hh)[perm] * scale[:, 0]

    wbd = np.zeros((P, 4 * P), np.float32)
    rj = np.zeros((2 * BL, 4 * P), np.float32)
    for g in range(4):
        Wg = Wh[g * 32 : (g + 1) * 32]    # (32, 32): [out_h, in_h]
        for b in range(BL):
            sl = slice(g * P + b * 32, g * P + b * 32 + 32)
            wbd[b * 32 : (b + 1) * 32, sl] = Wg.T
            rj[2 * b, sl] = Wi[g * 32 : (g + 1) * 32]
            rj[2 * b + 1, sl] = bs[g * 32 : (g + 1) * 32]

    wo = np.zeros((P, BL), np.float32)
    for b in range(BL):
        wo[b * 32 : (b + 1) * 32, b] = W_out[0]
    bo = np.full((BL, 1), np.float32(b_out[0]), np.float32)
    return wbd, rj, wo, bo


_NC_CACHE = {}


def _get_nc():
    key = (K, J)
    if key not in _NC_CACHE:
        _NC_CACHE[key] = build_nc(K, J, T)
    return _NC_CACHE[key]


def kernel(x, W_ih, W_hh, b_ih, b_hh, W_out, b_out):
    x = np.asarray(x, np.float32)
    wbd, rj, wo, bo = _host_precompute(
        np.asarray(W_ih, np.float32), np.asarray(W_hh, np.float32),
        np.asarray(b_ih, np.float32), np.asarray(b_hh, np.float32),
        np.asarray(W_out, np.float32), np.asarray(b_out, np.float32))

    xs = x[:, :, 0]                      # (B, T)
    in_maps = []
    for cidx in range(NCORES):
        in_maps.append({
            "x": np.ascontiguousarray(xs[cidx * BL : (cidx + 1) * BL]),
            "wbd": wbd, "rj": rj, "wo": wo, "bo": bo,
        })

    nc = _get_nc()
    res = run_bass_kernel_spmd(nc, in_maps, core_ids=list(range(NCORES)))
    ys = [res.results[cidx]["y"] for cidx in range(NCORES)]
    y = np.concatenate(ys, axis=0)       # (B, T)
    return y[:, :, None].astype(np.float32)
